# revision 15
# baseline (speedup 1.0000x reference)
"""AnaXnetGCN Trainium2 kernel — 8-core data parallel, fp16 compute.

Per core: 256 batches (padded to 259 = 37 groups x 7 batches x 18 nodes = 126 rows).
Software-pipelined over groups so TensorE keeps matmul work during the
softmax / leaky / layernorm vector chains of the previous group.

Math (per 126-row group, all matmuls fp16 in / f32 PSUM accumulate):
  u2  = (adj @ f) @ w1 = f' @ w1     lhsT = f'T chunks (f' folded on host)
  v   = leaky_relu(u2)               ACT copy + DVE max(0.2v, v)
  tT  = (blockdiag(adj) @ v)^T       lhsT = v chunks, rhs = blockdiag(adjT)
  x   = t @ w2                       lhsT = tT chunks, rhs = w2
  xT  = transpose(x)                 PE transposes
  S   = f @ x^T (per batch)          lhsT = fT chunks, rhs = xT chunks
  A   = softmax(S + blockmask)       stays block-diagonal
  an  = blockdiag(A) @ f             lhsT = A^T (PE transpose), rhs = f
  y   = an + f; z = LN(y)            bn_stats/bn_aggr; LN affine folded into fc
  fc  = wout^T @ z^T                 [48|126]: node head cols 0:14, graph 32:46
  nl  = transpose(fc[0:14]) -> out_nl;  gl = mean_n(fc[32:46]) -> out_gl
"""

import os
import sys

import numpy as np

if "/opt/trn_rl_repo" not in sys.path:
    sys.path.insert(0, "/opt/trn_rl_repo")

DEBUG_DUMP = bool(os.environ.get("BASS_GCN_DEBUG"))
B, N, D_IN, D_HID, D_OUT, C = 2048, 18, 1024, 2048, 1024, 14
NCORES = 8
BLOC = B // NCORES          # 256
G = 7                       # batches per group (7*18 = 126 rows <= 128)
GR = G * N                  # 126
NG = (BLOC + G - 1) // G    # 37
BPAD = NG * G               # 259
ROWS = BPAD * N             # 4662
KI = D_IN // 128            # 8
KH = D_HID // 128           # 16
KO = D_OUT // 128           # 8


def _build_graph():
    import concourse.bacc as bacc
    import concourse.mybir as mybir
    import concourse.tile as tile
    from contextlib import ExitStack

    f16 = mybir.dt.float16
    f32 = mybir.dt.float32

    nc = bacc.Bacc(
        "TRN2", target_bir_lowering=False, debug=False, num_devices=NCORES
    )
    pfT = nc.declare_dram_parameter("fT", [D_IN, ROWS], f16, isOutput=False)
    pfpT = nc.declare_dram_parameter("fpT", [D_IN, ROWS], f16, isOutput=False)
    pfn = nc.declare_dram_parameter("fnat", [ROWS, D_IN], f16, isOutput=False)
    pw1 = nc.declare_dram_parameter("w1", [D_IN, D_HID], f16, isOutput=False)
    pw2 = nc.declare_dram_parameter("w2", [D_HID, D_OUT], f16, isOutput=False)
    pwo = nc.declare_dram_parameter("wout", [D_OUT, 48], f16, isOutput=False)
    padj = nc.declare_dram_parameter("adjbd", [128, 128], f16, isOutput=False)
    pid = nc.declare_dram_parameter("ident", [128, 128], f16, isOutput=False)
    pmask = nc.declare_dram_parameter("mask", [128, 128], f32, isOutput=False)
    pbn = nc.declare_dram_parameter("biasn", [128, C], f32, isOutput=False)
    pbg = nc.declare_dram_parameter("biasg", [C, 1], f32, isOutput=False)
    onl = nc.declare_dram_parameter("out_nl", [ROWS, C], f32, isOutput=True)
    ogl = nc.declare_dram_parameter("out_gl", [C, BPAD], f32, isOutput=True)

    with tile.TileContext(nc) as tc, ExitStack() as ctx:
        consts = ctx.enter_context(tc.tile_pool(name="consts", bufs=1))
        io = ctx.enter_context(tc.tile_pool(name="io", bufs=3))
        work = ctx.enter_context(tc.tile_pool(name="work", bufs=2))
        small = ctx.enter_context(tc.tile_pool(name="small", bufs=3))
        ps512 = ctx.enter_context(tc.tile_pool(name="ps512", bufs=3, space="PSUM"))
        ps128 = ctx.enter_context(tc.tile_pool(name="ps128", bufs=5, space="PSUM"))

        # --- persistent constants / weights (per-chunk DMAs: fine-grained
        # deps so the first M1 matmuls start as soon as chunk 0 lands) ---
        adj_s = consts.tile([128, 128], f16)
        nc.sync.dma_start(out=adj_s, in_=padj[:, :])
        id_s = consts.tile([128, 128], f16)
        nc.sync.dma_start(out=id_s, in_=pid[:, :])
        mask_s = consts.tile([128, 128], f32)
        nc.sync.dma_start(out=mask_s, in_=pmask[:, :])
        eps_s = consts.tile([128, 1], f32)
        nc.vector.memset(eps_s, 1e-5)
        gl_acc = consts.tile([C, BPAD], f32)
        w1r = pw1[:, :].rearrange("(k p) h -> p k h", p=128)
        w1_s = consts.tile([128, KI, D_HID], f16)
        for k in range(KI):
            for hh in range(4):
                nc.sync.dma_start(
                    out=w1_s[:, k, hh * 512 : (hh + 1) * 512],
                    in_=w1r[:, k, hh * 512 : (hh + 1) * 512],
                )
        w2r = pw2[:, :].rearrange("(k p) d -> p k d", p=128)
        w2_s = consts.tile([128, KH, D_OUT], f16)
        wo_s = consts.tile([128, KO, 48], f16)
        biasn_s = consts.tile([128, C], f32)
        biasg_s = consts.tile([C, 1], f32)

        def load_late_consts():
            for k in range(KH):
                nc.sync.dma_start(out=w2_s[:, k, :], in_=w2r[:, k, :])
            nc.sync.dma_start(
                out=wo_s, in_=pwo[:, :].rearrange("(k p) c -> p k c", p=128)
            )
            nc.sync.dma_start(out=biasn_s, in_=pbn[:, :])
            nc.sync.dma_start(out=biasg_s, in_=pbg[:, :])

        st = [None] * NG  # per-group live state

        def front_a(g):
            """Loads; M1 (f' @ w1 -> u2 psum) with fused leaky -> v."""
            rs = g * GR
            fT_t = io.tile([128, KI, GR], f16, tag="fT")
            nc.sync.dma_start(
                out=fT_t,
                in_=pfT[:, rs : rs + GR].rearrange("(k p) r -> p k r", p=128),
            )
            fpT_t = io.tile([128, KI, GR], f16, tag="fpT")
            fpr = pfpT[:, rs : rs + GR].rearrange("(k p) r -> p k r", p=128)
            if g == 0:
                for k in range(KI):
                    nc.sync.dma_start(out=fpT_t[:, k, :], in_=fpr[:, k, :])
            else:
                nc.sync.dma_start(out=fpT_t, in_=fpr)
            fn_t = io.tile([GR, D_IN], f16, tag="fn")
            nc.sync.dma_start(out=fn_t, in_=pfn[rs : rs + GR, :])

            v_s = work.tile([GR, 4, 512], f16, tag="v")
            for h in range(4):
                pu = ps512.tile([128, 512], f32, tag="mm512")
                for k in range(KI):
                    nc.tensor.matmul(
                        pu[:GR],
                        fpT_t[:, k, :],
                        w1_s[:, k, h * 512 : (h + 1) * 512],
                        start=(k == 0),
                        stop=(k == KI - 1),
                    )
                nc.scalar.activation(
                    v_s[:, h, :], pu[:GR],
                    mybir.ActivationFunctionType.Relu, scale=0.8,
                )
                nc.vector.scalar_tensor_tensor(
                    v_s[:, h, :], pu[:GR], 0.2, v_s[:, h, :],
                    op0=mybir.AluOpType.mult, op1=mybir.AluOpType.add,
                )
            st[g] = {"fT": fT_t, "fn": fn_t, "v": v_s}

        def front_t(g):
            """tT = (blockdiag(adj) @ v)^T via fold."""
            s = st[g]
            v_flat = s["v"].rearrange("p h f -> p (h f)")
            tT_s = work.tile([128, KH, GR], f16, tag="tT")
            for k in range(KH):
                ptT = ps128.tile([128, 128], f32, tag="small")
                nc.tensor.matmul(
                    ptT[:, :GR],
                    v_flat[:, k * 128 : (k + 1) * 128],
                    adj_s[:GR, :GR],
                    start=True,
                    stop=True,
                )
                if k % 2 == 0:
                    nc.scalar.copy(tT_s[:, k, :], ptT[:, :GR])
                else:
                    nc.vector.tensor_copy(tT_s[:, k, :], ptT[:, :GR])
            s["tT"] = tT_s

        def front_b(g):
            """M2, xT, S matmuls + softmax vector chain."""
            s = st[g]
            tT_s = s["tT"]
            x_s = work.tile([GR, D_OUT], f16, tag="x")
            for h in range(2):
                px = ps512.tile([128, 512], f32, tag="mm512")
                for k in range(KH):
                    nc.tensor.matmul(
                        px[:GR],
                        tT_s[:, k, :],
                        w2_s[:, k, h * 512 : (h + 1) * 512],
                        start=(k == 0),
                        stop=(k == KH - 1),
                    )
                if h == 0:
                    nc.scalar.copy(x_s[:, h * 512 : (h + 1) * 512], px[:GR])
                else:
                    nc.vector.tensor_copy(x_s[:, h * 512 : (h + 1) * 512], px[:GR])

            xT_s = work.tile([128, KO, GR], f16, tag="xT")
            for k in range(KO):
                pxt = ps128.tile([128, 128], f16, tag="small")
                nc.tensor.matmul(
                    pxt[:, :GR],
                    x_s[:, k * 128 : (k + 1) * 128],
                    id_s[:GR, :GR],
                    start=True, stop=True, is_transpose=True,
                )
                if k % 2 == 0:
                    nc.scalar.copy(xT_s[:, k, :], pxt[:, :GR])
                else:
                    nc.vector.tensor_copy(xT_s[:, k, :], pxt[:, :GR])

            pS = ps128.tile([128, 128], f32, tag="small")
            for k in range(KO):
                nc.tensor.matmul(
                    pS[:GR, :GR],
                    s["fT"][:, k, :],
                    xT_s[:, k, :],
                    start=(k == 0),
                    stop=(k == KO - 1),
                )
            # softmax (block-diagonal attention), normalization deferred to scale
            Sm = small.tile([GR, GR], f32, tag="Sm")
            nc.vector.tensor_add(Sm, pS[:GR, :GR], mask_s[:GR, :GR])
            rmax = small.tile([GR, 1], f32, tag="rmax")
            nc.vector.reduce_max(rmax, Sm, axis=mybir.AxisListType.X)
            nmax = small.tile([GR, 1], f32, tag="nmax")
            nc.vector.tensor_scalar_mul(nmax, rmax, -1.0)
            E = small.tile([GR, GR], f16, tag="E")
            rsum = small.tile([GR, 1], f32, tag="rsum")
            nc.scalar.activation(
                E, Sm, mybir.ActivationFunctionType.Exp, bias=nmax, accum_out=rsum
            )
            rinv = small.tile([GR, 1], f32, tag="rinv")
            nc.vector.reciprocal(rinv, rsum)
            attn = small.tile([GR, GR], f16, tag="attn")
            nc.vector.tensor_scalar_mul(attn, E, rinv)
            s["attn"] = attn

        def back_a(g):
            """attnT, anatomy matmuls, residual + LN stats chain."""
            s = st[g]
            pAT = ps128.tile([128, 128], f16, tag="small")
            nc.tensor.matmul(
                pAT[:GR, :GR], s["attn"], id_s[:GR, :GR],
                start=True, stop=True, is_transpose=True,
            )
            attnT = small.tile([GR, GR], f16, tag="attnT")
            nc.scalar.copy(attnT, pAT[:GR, :GR])

            fn_t = s["fn"]
            y_s = work.tile([GR, D_IN], f16, tag="y")
            stats = small.tile([GR, 2, 6], f32, tag="stats")
            for h in range(2):
                pan = ps512.tile([128, 512], f32, tag="mm512")
                nc.tensor.matmul(
                    pan[:GR], attnT, fn_t[:, h * 512 : (h + 1) * 512],
                    start=True, stop=True,
                )
                nc.vector.tensor_add(
                    y_s[:, h * 512 : (h + 1) * 512],
                    pan[:GR],
                    fn_t[:, h * 512 : (h + 1) * 512],
                )
                nc.vector.bn_stats(stats[:, h, :], y_s[:, h * 512 : (h + 1) * 512])
            mv = small.tile([GR, 2], f32, tag="mv")
            nc.vector.bn_aggr(mv, stats)
            sd = small.tile([GR, 1], f32, tag="sd")
            nc.scalar.activation(
                sd, mv[:, 1:2], mybir.ActivationFunctionType.Sqrt, bias=eps_s[:GR]
            )
            rstd = small.tile([GR, 1], f32, tag="rstd")
            nc.vector.reciprocal(rstd, sd)
            z_s = work.tile([GR, D_IN], f16, tag="z")
            nc.vector.tensor_scalar(
                z_s, y_s, mv[:, 0:1], rstd,
                op0=mybir.AluOpType.subtract, op1=mybir.AluOpType.mult,
            )
            s["z"] = z_s

        def back_b(g):
            """zT transposes, fc matmuls, gl reduce, nl transpose + DMA out."""
            s = st[g]
            rs = g * GR
            z_s = s["z"]
            zT_s = work.tile([128, KO, GR], f16, tag="zT")
            for k in range(KO):
                pzt = ps128.tile([128, 128], f16, tag="small")
                nc.tensor.matmul(
                    pzt[:, :GR],
                    z_s[:, k * 128 : (k + 1) * 128],
                    id_s[:GR, :GR],
                    start=True, stop=True, is_transpose=True,
                )
                if k % 2 == 0:
                    nc.scalar.copy(zT_s[:, k, :], pzt[:, :GR])
                else:
                    nc.vector.tensor_copy(zT_s[:, k, :], pzt[:, :GR])
            pfc = ps128.tile([128, 128], f32, tag="small")
            for k in range(KO):
                nc.tensor.matmul(
                    pfc[:48, :GR],
                    wo_s[:, k, :],
                    zT_s[:, k, :],
                    start=(k == 0),
                    stop=(k == KO - 1),
                )
            nc.vector.reduce_sum(
                gl_acc[:, g * G : (g + 1) * G],
                pfc[32 : 32 + C, :GR].rearrange("c (b n) -> c b n", n=N),
                axis=mybir.AxisListType.X,
            )
            fcS = small.tile([C, GR], f16, tag="fcS")
            nc.scalar.copy(fcS, pfc[:C, :GR])
            pnl = ps128.tile([128, 128], f16, tag="small")
            nc.tensor.matmul(
                pnl[:GR, :C], fcS, id_s[:C, :C],
                start=True, stop=True, is_transpose=True,
            )
            nl_s = io.tile([GR, C], f32, tag="nl")
            nc.vector.tensor_add(nl_s, pnl[:GR, :C], biasn_s[:GR, :])
            nc.sync.dma_start(out=onl[rs : rs + GR, :], in_=nl_s)
            st[g] = None

        # --- software pipeline: PE always has group-g matmuls while group
        # g-1 runs its softmax / LN chains on DVE/ACT ---
        front_a(0)
        load_late_consts()
        front_t(0)
        front_b(0)
        for g in range(1, NG):
            front_a(g)      # M1(g) covers softmax(g-1)
            back_a(g - 1)   # attnT/an; LN chain issues on DVE
            front_t(g)      # tT(g) covers LN(g-1) chain
            back_b(g - 1)   # zT/fc/nl once z(g-1) ready
            front_b(g)      # M2/xT/S + softmax(g) vector ops
        back_a(NG - 1)
        back_b(NG - 1)

        # ---- final: graph logits mean + bias, transposed DMA out ----
        gl_fin = consts.tile([C, BPAD], f32)
        nc.vector.tensor_scalar(
            gl_fin, gl_acc, 1.0 / N, biasg_s,
            op0=mybir.AluOpType.mult, op1=mybir.AluOpType.add,
        )
        nc.sync.dma_start(out=ogl[:, :], in_=gl_fin)

    nc.compile()
    return nc


def kernel(feature, adj, w1, w2, ln_w, ln_b, fcn_w, fcn_b, fcg_w, fcg_b):
    from concourse.bass_utils import run_bass_kernel_spmd

    f32, f16 = np.float32, np.float16
    feature = np.asarray(feature, f32)
    adj = np.asarray(adj, f32)
    ln_w = np.asarray(ln_w, f32)
    ln_b = np.asarray(ln_b, f32)
    fcn_w = np.asarray(fcn_w, f32)
    fcn_b = np.asarray(fcn_b, f32)
    fcg_w = np.asarray(fcg_w, f32)
    fcg_b = np.asarray(fcg_b, f32)

    # fold LayerNorm affine into the output projections
    Wn = (fcn_w * ln_w[None, :]).T          # [1024,14]
    Wg = (fcg_w * ln_w[None, :]).T
    wout = np.zeros((D_OUT, 48), f16)       # node head cols 0:14, graph 32:46
    wout[:, :C] = Wn.astype(f16)
    wout[:, 32 : 32 + C] = Wg.astype(f16)
    bn_ = fcn_b + fcn_w @ ln_b
    bg_ = fcg_b + fcg_w @ ln_b
    biasn = np.ascontiguousarray(np.broadcast_to(bn_, (128, C)), dtype=f32)
    biasg = np.ascontiguousarray(bg_.reshape(C, 1), dtype=f32)

    adjbd = np.zeros((128, 128), f16)
    mask = np.full((128, 128), -30000.0, f32)
    for g in range(G):
        adjbd[g * N : (g + 1) * N, g * N : (g + 1) * N] = adj.T.astype(f16)
        mask[g * N : (g + 1) * N, g * N : (g + 1) * N] = 0.0
    ident = np.eye(128, dtype=f16)

    # first GCN adjacency application folded onto the host: f' = adj @ f
    fprime = np.einsum("ij,bjd->bid", adj, feature).astype(f16)

    fpad = np.zeros((NCORES, BPAD, N, D_IN), f16)
    fpad[:, :BLOC] = feature.reshape(NCORES, BLOC, N, D_IN).astype(f16)
    fnat = np.ascontiguousarray(fpad.reshape(NCORES, ROWS, D_IN))
    fT = np.ascontiguousarray(fnat.transpose(0, 2, 1))
    fppad = np.zeros((NCORES, BPAD, N, D_IN), f16)
    fppad[:, :BLOC] = fprime.reshape(NCORES, BLOC, N, D_IN)
    fpT = np.ascontiguousarray(fppad.reshape(NCORES, ROWS, D_IN).transpose(0, 2, 1))

    w1h = np.asarray(w1, f32).astype(f16)
    w2h = np.asarray(w2, f32).astype(f16)

    nc = _build_graph()
    in_maps = []
    for i in range(NCORES):
        in_maps.append(
            {
                "fT": fT[i],
                "fpT": fpT[i],
                "fnat": fnat[i],
                "w1": w1h,
                "w2": w2h,
                "wout": wout,
                "adjbd": adjbd,
                "ident": ident,
                "mask": mask,
                "biasn": biasn,
                "biasg": biasg,
            }
        )

    trace = bool(os.environ.get("BASS_GCN_TRACE"))
    repeat = int(os.environ.get("BASS_GCN_REPEAT", "1"))
    times = []
    for _ in range(repeat):
        res = run_bass_kernel_spmd(
            nc, in_maps, core_ids=list(range(NCORES)), trace=trace
        )
        if trace:
            times.append(res.exec_time_ns)
            kernel.last_exec_time_ns = res.exec_time_ns
            kernel.last_profile = res
    kernel.exec_times = times
    outs = res.results

    node = np.empty((B, N, C), f32)
    graph = np.empty((B, C), f32)
    for i in range(NCORES):
        node[i * BLOC : (i + 1) * BLOC] = (
            outs[i]["out_nl"][: BLOC * N].reshape(BLOC, N, C)
        )
        graph[i * BLOC : (i + 1) * BLOC] = outs[i]["out_gl"].T[:BLOC]
    return node, graph


# revision 16
# speedup vs baseline: 1.2050x; 1.2050x over previous
"""AnaXnetGCN Trainium2 kernel — 8-core data parallel, fp16 compute.

Per core: 256 batches (padded to 259 = 37 groups x 7 batches x 18 nodes = 126 rows).
Software-pipelined over groups so TensorE keeps matmul work during the
softmax / leaky / layernorm vector chains of the previous group.

Math (per 126-row group, all matmuls fp16 in / f32 PSUM accumulate):
  u2  = (adj @ f) @ w1 = f' @ w1     lhsT = f'T chunks (f' folded on host)
  v   = leaky_relu(u2)               ACT copy + DVE max(0.2v, v)
  tT  = (blockdiag(adj) @ v)^T       lhsT = v chunks, rhs = blockdiag(adjT)
  x   = t @ w2                       lhsT = tT chunks, rhs = w2
  xT  = transpose(x)                 PE transposes
  S   = f @ x^T (per batch)          lhsT = fT chunks, rhs = xT chunks
  A   = softmax(S + blockmask)       stays block-diagonal
  an  = blockdiag(A) @ f             lhsT = A^T (PE transpose), rhs = f
  y   = an + f; z = LN(y)            bn_stats/bn_aggr; LN affine folded into fc
  fc  = wout^T @ z^T                 [48|126]: node head cols 0:14, graph 32:46
  nl  = transpose(fc[0:14]) -> out_nl;  gl = mean_n(fc[32:46]) -> out_gl
"""

import os
import sys

import numpy as np

if "/opt/trn_rl_repo" not in sys.path:
    sys.path.insert(0, "/opt/trn_rl_repo")

DEBUG_DUMP = bool(os.environ.get("BASS_GCN_DEBUG"))
B, N, D_IN, D_HID, D_OUT, C = 2048, 18, 1024, 2048, 1024, 14
NCORES = 8
BLOC = B // NCORES          # 256
G = 7                       # batches per group (7*18 = 126 rows <= 128)
GR = G * N                  # 126
NG = (BLOC + G - 1) // G    # 37
BPAD = NG * G               # 259
ROWS = BPAD * N             # 4662
KI = D_IN // 128            # 8
KH = D_HID // 128           # 16
KO = D_OUT // 128           # 8


def _build_graph():
    import concourse.bacc as bacc
    import concourse.mybir as mybir
    import concourse.tile as tile
    from contextlib import ExitStack

    f16 = mybir.dt.float16
    f32 = mybir.dt.float32

    nc = bacc.Bacc(
        "TRN2", target_bir_lowering=False, debug=False, num_devices=NCORES
    )
    pfT = nc.declare_dram_parameter("fT", [D_IN, ROWS], f16, isOutput=False)
    pfpT = nc.declare_dram_parameter("fpT", [D_IN, ROWS], f16, isOutput=False)
    pfn = nc.declare_dram_parameter("fnat", [ROWS, D_IN], f16, isOutput=False)
    pw1 = nc.declare_dram_parameter("w1", [D_IN, D_HID], f16, isOutput=False)
    pw2 = nc.declare_dram_parameter("w2", [D_HID, D_OUT], f16, isOutput=False)
    pwo = nc.declare_dram_parameter("wout", [D_OUT, 48], f16, isOutput=False)
    padj = nc.declare_dram_parameter("adjbd", [128, 128], f16, isOutput=False)
    pid = nc.declare_dram_parameter("ident", [128, 128], f16, isOutput=False)
    pmask = nc.declare_dram_parameter("mask", [128, 128], f32, isOutput=False)
    pbn = nc.declare_dram_parameter("biasn", [128, C], f32, isOutput=False)
    pbg = nc.declare_dram_parameter("biasg", [C, 1], f32, isOutput=False)
    onl = nc.declare_dram_parameter("out_nl", [ROWS, C], f32, isOutput=True)
    ogl = nc.declare_dram_parameter("out_gl", [C, BPAD], f32, isOutput=True)

    with tile.TileContext(nc) as tc, ExitStack() as ctx:
        consts = ctx.enter_context(tc.tile_pool(name="consts", bufs=1))
        io = ctx.enter_context(tc.tile_pool(name="io", bufs=3))
        work = ctx.enter_context(tc.tile_pool(name="work", bufs=2))
        small = ctx.enter_context(tc.tile_pool(name="small", bufs=3))
        ps512 = ctx.enter_context(tc.tile_pool(name="ps512", bufs=3, space="PSUM"))
        ps128 = ctx.enter_context(tc.tile_pool(name="ps128", bufs=5, space="PSUM"))

        # --- persistent constants / weights (per-chunk DMAs: fine-grained
        # deps so the first M1 matmuls start as soon as chunk 0 lands) ---
        _g0_fpT = io.tile([128, KI, GR], f16, tag="fpT")
        _fpr0 = pfpT[:, 0:GR].rearrange("(k p) r -> p k r", p=128)
        for _k in range(KI):
            nc.sync.dma_start(out=_g0_fpT[:, _k, :], in_=_fpr0[:, _k, :])
        _g0_fT = io.tile([128, KI, GR], f16, tag="fT")
        nc.sync.dma_start(
            out=_g0_fT, in_=pfT[:, 0:GR].rearrange("(k p) r -> p k r", p=128)
        )
        _g0_fn = io.tile([GR, D_IN], f16, tag="fn")
        nc.sync.dma_start(out=_g0_fn, in_=pfn[0:GR, :])
        _g0 = (_g0_fpT, _g0_fT, _g0_fn)
        adj_s = consts.tile([128, 128], f16)
        nc.gpsimd.dma_start(out=adj_s, in_=padj[:, :])
        id_s = consts.tile([128, 128], f16)
        nc.gpsimd.dma_start(out=id_s, in_=pid[:, :])
        mask_s = consts.tile([128, 128], f32)
        nc.gpsimd.dma_start(out=mask_s, in_=pmask[:, :])
        eps_s = consts.tile([128, 1], f32)
        nc.vector.memset(eps_s, 1e-5)
        gl_acc = consts.tile([C, BPAD], f32)
        w1r = pw1[:, :].rearrange("(k p) h -> p k h", p=128)
        w1_s = consts.tile([128, KI, D_HID], f16)
        for k in range(KI):
            for hh in range(4):
                nc.gpsimd.dma_start(
                    out=w1_s[:, k, hh * 512 : (hh + 1) * 512],
                    in_=w1r[:, k, hh * 512 : (hh + 1) * 512],
                )
        w2r = pw2[:, :].rearrange("(k p) d -> p k d", p=128)
        w2_s = consts.tile([128, KH, D_OUT], f16)
        wo_s = consts.tile([128, KO, 48], f16)
        biasn_s = consts.tile([128, C], f32)
        biasg_s = consts.tile([C, 1], f32)

        def load_late_consts():
            for k in range(KH):
                nc.gpsimd.dma_start(out=w2_s[:, k, :], in_=w2r[:, k, :])
            nc.gpsimd.dma_start(
                out=wo_s, in_=pwo[:, :].rearrange("(k p) c -> p k c", p=128)
            )
            nc.gpsimd.dma_start(out=biasn_s, in_=pbn[:, :])
            nc.gpsimd.dma_start(out=biasg_s, in_=pbg[:, :])

        st = [None] * NG  # per-group live state

        def load_group(g):
            rs = g * GR
            fpT_t = io.tile([128, KI, GR], f16, tag="fpT")
            fpr = pfpT[:, rs : rs + GR].rearrange("(k p) r -> p k r", p=128)
            if g == 0:
                for k in range(KI):
                    nc.sync.dma_start(out=fpT_t[:, k, :], in_=fpr[:, k, :])
            else:
                nc.sync.dma_start(out=fpT_t, in_=fpr)
            fT_t = io.tile([128, KI, GR], f16, tag="fT")
            nc.sync.dma_start(
                out=fT_t,
                in_=pfT[:, rs : rs + GR].rearrange("(k p) r -> p k r", p=128),
            )
            fn_t = io.tile([GR, D_IN], f16, tag="fn")
            nc.sync.dma_start(out=fn_t, in_=pfn[rs : rs + GR, :])
            return fpT_t, fT_t, fn_t

        def front_a(g, pre=None):
            """M1 (f' @ w1 -> u2 psum) with fused leaky -> v."""
            fpT_t, fT_t, fn_t = pre if pre is not None else load_group(g)

            v_s = work.tile([GR, 4, 512], f16, tag="v")
            for h in range(4):
                pu = ps512.tile([128, 512], f32, tag="mm512")
                for k in range(KI):
                    nc.tensor.matmul(
                        pu[:GR],
                        fpT_t[:, k, :],
                        w1_s[:, k, h * 512 : (h + 1) * 512],
                        start=(k == 0),
                        stop=(k == KI - 1),
                    )
                nc.scalar.activation(
                    v_s[:, h, :], pu[:GR],
                    mybir.ActivationFunctionType.Relu, scale=0.8,
                )
                nc.vector.scalar_tensor_tensor(
                    v_s[:, h, :], pu[:GR], 0.2, v_s[:, h, :],
                    op0=mybir.AluOpType.mult, op1=mybir.AluOpType.add,
                )
            st[g] = {"fT": fT_t, "fn": fn_t, "v": v_s}

        def front_t(g):
            """tT = (blockdiag(adj) @ v)^T via fold."""
            s = st[g]
            v_flat = s["v"].rearrange("p h f -> p (h f)")
            tT_s = work.tile([128, KH, GR], f16, tag="tT")
            for k in range(KH):
                ptT = ps128.tile([128, 128], f32, tag="small")
                nc.tensor.matmul(
                    ptT[:, :GR],
                    v_flat[:, k * 128 : (k + 1) * 128],
                    adj_s[:GR, :GR],
                    start=True,
                    stop=True,
                )
                if k % 2 == 0:
                    nc.scalar.copy(tT_s[:, k, :], ptT[:, :GR])
                else:
                    nc.vector.tensor_copy(tT_s[:, k, :], ptT[:, :GR])
            s["tT"] = tT_s

        def front_b(g):
            """M2, xT, S matmuls + softmax vector chain."""
            s = st[g]
            tT_s = s["tT"]
            x_s = work.tile([GR, D_OUT], f16, tag="x")
            for h in range(2):
                px = ps512.tile([128, 512], f32, tag="mm512")
                for k in range(KH):
                    nc.tensor.matmul(
                        px[:GR],
                        tT_s[:, k, :],
                        w2_s[:, k, h * 512 : (h + 1) * 512],
                        start=(k == 0),
                        stop=(k == KH - 1),
                    )
                if h == 0:
                    nc.scalar.copy(x_s[:, h * 512 : (h + 1) * 512], px[:GR])
                else:
                    nc.vector.tensor_copy(x_s[:, h * 512 : (h + 1) * 512], px[:GR])

            xT_s = work.tile([128, KO, GR], f16, tag="xT")
            for k in range(KO):
                pxt = ps128.tile([128, 128], f16, tag="small")
                nc.tensor.matmul(
                    pxt[:, :GR],
                    x_s[:, k * 128 : (k + 1) * 128],
                    id_s[:GR, :GR],
                    start=True, stop=True, is_transpose=True,
                )
                if k % 2 == 0:
                    nc.scalar.copy(xT_s[:, k, :], pxt[:, :GR])
                else:
                    nc.vector.tensor_copy(xT_s[:, k, :], pxt[:, :GR])

            pS = ps128.tile([128, 128], f32, tag="small")
            for k in range(KO):
                nc.tensor.matmul(
                    pS[:GR, :GR],
                    s["fT"][:, k, :],
                    xT_s[:, k, :],
                    start=(k == 0),
                    stop=(k == KO - 1),
                )
            # softmax (block-diagonal attention), normalization deferred to scale
            Sm = small.tile([GR, GR], f32, tag="Sm")
            nc.vector.tensor_add(Sm, pS[:GR, :GR], mask_s[:GR, :GR])
            rmax = small.tile([GR, 1], f32, tag="rmax")
            nc.vector.reduce_max(rmax, Sm, axis=mybir.AxisListType.X)
            nmax = small.tile([GR, 1], f32, tag="nmax")
            nc.vector.tensor_scalar_mul(nmax, rmax, -1.0)
            E = small.tile([GR, GR], f16, tag="E")
            rsum = small.tile([GR, 1], f32, tag="rsum")
            nc.scalar.activation(
                E, Sm, mybir.ActivationFunctionType.Exp, bias=nmax, accum_out=rsum
            )
            rinv = small.tile([GR, 1], f32, tag="rinv")
            nc.vector.reciprocal(rinv, rsum)
            attn = small.tile([GR, GR], f16, tag="attn")
            nc.vector.tensor_scalar_mul(attn, E, rinv)
            s["attn"] = attn

        def back_a(g):
            """attnT, anatomy matmuls, residual + LN stats chain."""
            s = st[g]
            pAT = ps128.tile([128, 128], f16, tag="small")
            nc.tensor.matmul(
                pAT[:GR, :GR], s["attn"], id_s[:GR, :GR],
                start=True, stop=True, is_transpose=True,
            )
            attnT = small.tile([GR, GR], f16, tag="attnT")
            nc.scalar.copy(attnT, pAT[:GR, :GR])

            fn_t = s["fn"]
            y_s = work.tile([GR, D_IN], f16, tag="y")
            stats = small.tile([GR, 2, 6], f32, tag="stats")
            for h in range(2):
                pan = ps512.tile([128, 512], f32, tag="mm512")
                nc.tensor.matmul(
                    pan[:GR], attnT, fn_t[:, h * 512 : (h + 1) * 512],
                    start=True, stop=True,
                )
                nc.vector.tensor_add(
                    y_s[:, h * 512 : (h + 1) * 512],
                    pan[:GR],
                    fn_t[:, h * 512 : (h + 1) * 512],
                )
                nc.vector.bn_stats(stats[:, h, :], y_s[:, h * 512 : (h + 1) * 512])
            mv = small.tile([GR, 2], f32, tag="mv")
            nc.vector.bn_aggr(mv, stats)
            sd = small.tile([GR, 1], f32, tag="sd")
            nc.scalar.activation(
                sd, mv[:, 1:2], mybir.ActivationFunctionType.Sqrt, bias=eps_s[:GR]
            )
            rstd = small.tile([GR, 1], f32, tag="rstd")
            nc.vector.reciprocal(rstd, sd)
            z_s = work.tile([GR, D_IN], f16, tag="z")
            nc.vector.tensor_scalar(
                z_s, y_s, mv[:, 0:1], rstd,
                op0=mybir.AluOpType.subtract, op1=mybir.AluOpType.mult,
            )
            s["z"] = z_s

        def back_b(g):
            """zT transposes, fc matmuls, gl reduce, nl transpose + DMA out."""
            s = st[g]
            rs = g * GR
            z_s = s["z"]
            zT_s = work.tile([128, KO, GR], f16, tag="zT")
            for k in range(KO):
                pzt = ps128.tile([128, 128], f16, tag="small")
                nc.tensor.matmul(
                    pzt[:, :GR],
                    z_s[:, k * 128 : (k + 1) * 128],
                    id_s[:GR, :GR],
                    start=True, stop=True, is_transpose=True,
                )
                if k % 2 == 0:
                    nc.scalar.copy(zT_s[:, k, :], pzt[:, :GR])
                else:
                    nc.vector.tensor_copy(zT_s[:, k, :], pzt[:, :GR])
            pfc = ps128.tile([128, 128], f32, tag="small")
            for k in range(KO):
                nc.tensor.matmul(
                    pfc[:48, :GR],
                    wo_s[:, k, :],
                    zT_s[:, k, :],
                    start=(k == 0),
                    stop=(k == KO - 1),
                )
            nc.vector.reduce_sum(
                gl_acc[:, g * G : (g + 1) * G],
                pfc[32 : 32 + C, :GR].rearrange("c (b n) -> c b n", n=N),
                axis=mybir.AxisListType.X,
            )
            fcS = small.tile([C, GR], f16, tag="fcS")
            nc.scalar.copy(fcS, pfc[:C, :GR])
            pnl = ps128.tile([128, 128], f16, tag="small")
            nc.tensor.matmul(
                pnl[:GR, :C], fcS, id_s[:C, :C],
                start=True, stop=True, is_transpose=True,
            )
            nl_s = io.tile([GR, C], f32, tag="nl")
            nc.vector.tensor_add(nl_s, pnl[:GR, :C], biasn_s[:GR, :])
            nc.sync.dma_start(out=onl[rs : rs + GR, :], in_=nl_s)
            st[g] = None

        # --- software pipeline: PE always has group-g matmuls while group
        # g-1 runs its softmax / LN chains on DVE/ACT ---
        front_a(0, pre=_g0)
        load_late_consts()
        front_t(0)
        front_b(0)
        for g in range(1, NG):
            front_a(g)      # M1(g) covers softmax(g-1)
            back_a(g - 1)   # attnT/an; LN chain issues on DVE
            front_t(g)      # tT(g) covers LN(g-1) chain
            back_b(g - 1)   # zT/fc/nl once z(g-1) ready
            front_b(g)      # M2/xT/S + softmax(g) vector ops
        back_a(NG - 1)
        back_b(NG - 1)

        # ---- final: graph logits mean + bias, transposed DMA out ----
        gl_fin = consts.tile([C, BPAD], f32)
        nc.vector.tensor_scalar(
            gl_fin, gl_acc, 1.0 / N, biasg_s,
            op0=mybir.AluOpType.mult, op1=mybir.AluOpType.add,
        )
        nc.sync.dma_start(out=ogl[:, :], in_=gl_fin)

    nc.compile()
    return nc


def kernel(feature, adj, w1, w2, ln_w, ln_b, fcn_w, fcn_b, fcg_w, fcg_b):
    from concourse.bass_utils import run_bass_kernel_spmd

    f32, f16 = np.float32, np.float16
    feature = np.asarray(feature, f32)
    adj = np.asarray(adj, f32)
    ln_w = np.asarray(ln_w, f32)
    ln_b = np.asarray(ln_b, f32)
    fcn_w = np.asarray(fcn_w, f32)
    fcn_b = np.asarray(fcn_b, f32)
    fcg_w = np.asarray(fcg_w, f32)
    fcg_b = np.asarray(fcg_b, f32)

    # fold LayerNorm affine into the output projections
    Wn = (fcn_w * ln_w[None, :]).T          # [1024,14]
    Wg = (fcg_w * ln_w[None, :]).T
    wout = np.zeros((D_OUT, 48), f16)       # node head cols 0:14, graph 32:46
    wout[:, :C] = Wn.astype(f16)
    wout[:, 32 : 32 + C] = Wg.astype(f16)
    bn_ = fcn_b + fcn_w @ ln_b
    bg_ = fcg_b + fcg_w @ ln_b
    biasn = np.ascontiguousarray(np.broadcast_to(bn_, (128, C)), dtype=f32)
    biasg = np.ascontiguousarray(bg_.reshape(C, 1), dtype=f32)

    adjbd = np.zeros((128, 128), f16)
    mask = np.full((128, 128), -30000.0, f32)
    for g in range(G):
        adjbd[g * N : (g + 1) * N, g * N : (g + 1) * N] = adj.T.astype(f16)
        mask[g * N : (g + 1) * N, g * N : (g + 1) * N] = 0.0
    ident = np.eye(128, dtype=f16)

    # first GCN adjacency application folded onto the host: f' = adj @ f
    fprime = np.einsum("ij,bjd->bid", adj, feature).astype(f16)

    fpad = np.zeros((NCORES, BPAD, N, D_IN), f16)
    fpad[:, :BLOC] = feature.reshape(NCORES, BLOC, N, D_IN).astype(f16)
    fnat = np.ascontiguousarray(fpad.reshape(NCORES, ROWS, D_IN))
    fT = np.ascontiguousarray(fnat.transpose(0, 2, 1))
    fppad = np.zeros((NCORES, BPAD, N, D_IN), f16)
    fppad[:, :BLOC] = fprime.reshape(NCORES, BLOC, N, D_IN)
    fpT = np.ascontiguousarray(fppad.reshape(NCORES, ROWS, D_IN).transpose(0, 2, 1))

    w1h = np.asarray(w1, f32).astype(f16)
    w2h = np.asarray(w2, f32).astype(f16)

    nc = _build_graph()
    in_maps = []
    for i in range(NCORES):
        in_maps.append(
            {
                "fT": fT[i],
                "fpT": fpT[i],
                "fnat": fnat[i],
                "w1": w1h,
                "w2": w2h,
                "wout": wout,
                "adjbd": adjbd,
                "ident": ident,
                "mask": mask,
                "biasn": biasn,
                "biasg": biasg,
            }
        )

    trace = bool(os.environ.get("BASS_GCN_TRACE"))
    repeat = int(os.environ.get("BASS_GCN_REPEAT", "1"))
    times = []
    for _ in range(repeat):
        res = run_bass_kernel_spmd(
            nc, in_maps, core_ids=list(range(NCORES)), trace=trace
        )
        if trace:
            times.append(res.exec_time_ns)
            kernel.last_exec_time_ns = res.exec_time_ns
            kernel.last_profile = res
    kernel.exec_times = times
    outs = res.results

    node = np.empty((B, N, C), f32)
    graph = np.empty((B, C), f32)
    for i in range(NCORES):
        node[i * BLOC : (i + 1) * BLOC] = (
            outs[i]["out_nl"][: BLOC * N].reshape(BLOC, N, C)
        )
        graph[i * BLOC : (i + 1) * BLOC] = outs[i]["out_gl"].T[:BLOC]
    return node, graph
